# revision 1
# baseline (speedup 1.0000x reference)
"""nn_AdapFilter3d Trainium2 kernel — 8-core SPMD (data-parallel over (B,C)).

out[b,c,z,y,x] = sum_{i,j,k} pad(input)[b,c,z+i-1,y+j-1,x+k-1] * F[b,c,z,y,x,i,j,k]

Strategy (per NeuronCore, 4 of the 32 independent (b,c) slices, 2 slice-pairs):
  - Partition layout p = 64*s_local + y; free dims carry (z, x) densely. No
    on-chip spatial halos: edge-tap F values are zeroed host-side, which is
    exact because those contributions are zero through the reference's zero
    padding of the input.
  - Host data layouts are y-major so every DMA moves one big contiguous run
    per partition (small DMA packets pay ~210ns fixed overhead each):
      F:     [S*H, D, 27*W] bf16, tap-major x-contiguous (full-rate VectorE
             reads); (s,y) merged so one 128-partition DMA loads a z-chunk
             with 27.6KB runs.
      input: [S, H+2, 65 | D*W | 65] bf16 with zero rows/pads baked in; the
             y-shift of each j tap is baked into three separately DMA'd
             copies (VectorE lanes cannot read across partitions).
      out:   [S, H, D, W] bf16, transposed/cast back on host.
  - Per (pair, z-chunk of 8): 27 VectorE multiplies (shifted dense window x
    F tap window, bf16 2x packed mode), each forwarded into PSUM by a
    TensorE identity matmul with accumulation — the otherwise-idle PE does
    all 26 adds in fp32. ScalarE evicts PSUM; DMA out.
  - Large DMAs alternate between the two HWDGE rings (sync/scalar) so both
    SDMA engine sets stream concurrently.

Measured on 8xTRN2 (neuron-profile): ~114 us HW exec, L2 rel err ~3.1e-3
(fp32 reference; bf16 input rounding dominates the error).

Self-contained: hardcodes shapes from the problem spec; needs only the
concourse/axon environment on sys.path.
"""

import time

import numpy as np

import concourse.bacc as bacc
import concourse.tile as tile
from concourse import mybir
from concourse.bass_utils import run_bass_kernel_spmd

B, C, D, H, W = 2, 16, 32, 64, 64
TAPS = 27
N_CORES = 8
S = (B * C) // N_CORES  # 4 slices per core
PAIRS = S // 2  # 2
ZC = 8  # z planes per chunk
NCHUNK = D // ZC  # 4
XT = W * TAPS  # 1728
FD = ZC * W  # 512
DW = D * W  # 2048 dense (z,x) elements per (slice, y)
FRONT = 65  # zero pad around the dense (z,x) block (>= W+1)
XPLEN = FRONT + DW + FRONT

F32 = mybir.dt.float32
IO_DT = mybir.dt.bfloat16


def _build(prod_dt=mybir.dt.bfloat16):
    nc = bacc.Bacc()
    x_ext = nc.declare_dram_parameter("input", [S, H + 2, XPLEN], IO_DT, isOutput=False)
    f_ext = nc.declare_dram_parameter("F", [S * H, D, XT], IO_DT, isOutput=False)
    id_ext = nc.declare_dram_parameter("ident", [128, 128], prod_dt, isOutput=False)
    out_ext = nc.declare_dram_parameter("out", [S, H, D, W], IO_DT, isOutput=True)

    with tile.TileContext(nc) as tc:
        with (
            tc.tile_pool(name="const", bufs=1) as cpool,
            tc.tile_pool(name="xp", bufs=6) as xpool,
            tc.tile_pool(name="fp", bufs=3) as fpool,
            tc.tile_pool(name="prod", bufs=4) as ppool,
            tc.tile_pool(name="osb", bufs=2) as opool,
            tc.tile_pool(name="ps", bufs=4, space="PSUM") as pspool,
        ):
            ident = cpool.tile([128, 128], prod_dt)
            nc.sync.dma_start(ident[:], id_ext[:])

            for pair in range(PAIRS):
                xps = []
                for j in range(3):
                    dy = j - 1
                    xp = xpool.tile([128, XPLEN], IO_DT, tag="xp")
                    # input rows are pre-padded on the host (zero rows at 0
                    # and H+1, zero front/back pads), so each y-shifted copy
                    # is one pure DMA per slice with no memset dependencies.
                    for s in range(2):
                        dma_eng = nc.sync if s == 0 else nc.scalar
                        sl = pair * 2 + s
                        dma_eng.dma_start(
                            xp[64 * s : 64 * s + 64, :],
                            x_ext[sl, dy + 1 : dy + 65, :],
                        )
                    xps.append(xp)

                for zc in range(NCHUNK):
                    ft = fpool.tile([128, ZC * XT], IO_DT, tag="ft")
                    ft3 = ft[:].rearrange("p (z q) -> p z q", z=ZC)
                    ft4 = ft[:].rearrange("p (z t x) -> p z t x", z=ZC, t=TAPS)
                    dma_eng = nc.sync if zc % 2 == 0 else nc.scalar
                    dma_eng.dma_start(
                        ft3[:, :, :],
                        f_ext[pair * 128 : (pair + 1) * 128, zc * ZC : (zc + 1) * ZC, :],
                    )
                    psumt = pspool.tile([128, FD], F32, tag="ps")
                    for t in range(TAPS):
                        i, j, k = t // 9, (t // 3) % 3, t % 3
                        prod = ppool.tile([128, FD], prod_dt, tag="prod")
                        prod3 = prod[:].rearrange("p (z x) -> p z x", z=ZC)
                        off0 = FRONT + (zc * ZC + i - 1) * W + (k - 1)
                        nc.vector.tensor_mul(
                            prod3[:, :, :],
                            xps[j][:, off0 : off0 + FD].rearrange(
                                "p (z x) -> p z x", z=ZC
                            ),
                            ft4[:, :, t, :],
                        )
                        nc.tensor.matmul(
                            psumt[:],
                            ident[:],
                            prod[:],
                            start=(t == 0),
                            stop=(t == TAPS - 1),
                        )
                    osb = opool.tile([128, FD], IO_DT, tag="osb")
                    nc.scalar.copy(osb[:], psumt[:])
                    osb3 = osb[:].rearrange("p (z x) -> p z x", z=ZC)
                    for s in range(2):
                        sl = pair * 2 + s
                        dma_eng = nc.scalar if s == 0 else nc.sync
                        dma_eng.dma_start(
                            out_ext[sl, :, zc * ZC : (zc + 1) * ZC, :],
                            osb3[64 * s : 64 * s + 64, :, :],
                        )
    nc.compile()
    return nc


_NC_CACHE = {}


def kernel(input: np.ndarray, F: np.ndarray) -> np.ndarray:
    input = np.asarray(input)
    F = np.asarray(F)
    assert input.shape == (B, C, D, H, W), input.shape
    assert F.shape == (B, C, D, H, W, 3, 3, 3), F.shape

    if "nc" not in _NC_CACHE:
        _NC_CACHE["nc"] = _build()
    nc = _NC_CACHE["nc"]

    io_np = mybir.dt.np(IO_DT)
    # padded input: [BC, H+2, FRONT | D*W | FRONT], zero rows at 0 and H+1
    xs = np.zeros((B * C, H + 2, XPLEN), dtype=io_np)
    xs[:, 1 : H + 1, FRONT : FRONT + DW] = (
        input.reshape(B * C, D, H, W)
        .transpose(0, 2, 1, 3)
        .reshape(B * C, H, DW)
        .astype(io_np)
    )
    # F -> [BC, H, D, TAPS, W] tap-major; zero edge-tap entries (exact: their
    # reference contribution is zero via the input's zero padding)
    ftp = np.ascontiguousarray(
        F.reshape(B * C, D, H, W, TAPS).transpose(0, 2, 1, 4, 3).astype(io_np)
    )
    tap = np.arange(TAPS)
    ftp[:, :, :, tap % 3 == 0, 0] = 0
    ftp[:, :, :, tap % 3 == 2, W - 1] = 0
    ftp[:, :, 0, tap // 9 == 0, :] = 0
    ftp[:, :, D - 1, tap // 9 == 2, :] = 0
    fs = ftp.reshape(B * C * H, D, XT)
    ident = np.eye(128, dtype=io_np)

    in_maps = [
        {
            "input": xs[c * S : (c + 1) * S],
            "F": fs[c * S * H : (c + 1) * S * H],
            "ident": ident,
        }
        for c in range(N_CORES)
    ]
    # the fleet occasionally throws transient NRT_EXEC_UNIT_UNRECOVERABLE
    # device errors (observed twice in dev, both cleared on retry)
    last_err = None
    for _attempt in range(3):
        try:
            res = run_bass_kernel_spmd(nc, in_maps, core_ids=list(range(N_CORES)))
            break
        except Exception as e:  # noqa: BLE001
            last_err = e
            time.sleep(2.0)
    else:
        raise last_err
    out = np.concatenate(
        [np.asarray(res.results[c]["out"], dtype=np.float32) for c in range(N_CORES)],
        axis=0,
    )  # [BC, H, D, W]
    return np.ascontiguousarray(
        out.transpose(0, 2, 1, 3).reshape(B, C, D, H, W).astype(np.float32)
    )

